# revision 8
# baseline (speedup 1.0000x reference)
"""Trainium2 Bass kernel for nn_DenseEncoderLayer (qk-norm attention + dense MoE).

Sharding: all 8 cores run one SPMD program. Attention + norms are computed
redundantly on every core; the MoE expert dimension (E=8) is sharded one
expert per core (gate columns permuted per-core so column 0 is the local
expert). The gate-weighted expert partials are combined with an on-device
ReduceScatter over the token axis; each core applies the final LayerNorm to
its token shard and the host concatenates the shards.

Matmuls run as float32r (full-rate fp32 on the PE array).
"""
import numpy as np

import concourse.bass as bass
import concourse.tile as tile
from concourse import bacc, mybir
from concourse.bass_utils import run_bass_kernel_spmd

F32 = mybir.dt.float32
F32R = mybir.dt.float32r
AX = mybir.AxisListType
OP = mybir.AluOpType
ACTF = mybir.ActivationFunctionType

# Problem dims (hardcoded per spec)
B, S, D, H, DH, E, FF = 2, 1024, 1024, 16, 64, 8, 4096
N = B * S                 # 2048 tokens
P = 128                   # partitions
TT = N // P               # 16 token tiles
DC = D // P               # 8 d-chunks
FC = FF // P              # 32 ff-chunks
N_CORES = 8
QK_SCALE = 10.0
NEG_SHIFT = -10.0         # exp(QK_SCALE*logit - 10); logits in [-1, 1]

_CACHE = {}


def _build(flags):
    """Build the SPMD Bass program. flags: (ln_affine, qs, gate_b, b2)"""
    ln_affine, use_qs, use_gb, use_b2 = flags
    nc = bacc.Bacc("TRN2", target_bir_lowering=False, debug=False,
                   enable_asserts=False, num_devices=N_CORES)

    din = {}
    def inp(name, shape):
        din[name] = nc.dram_tensor(name, list(shape), F32, kind="ExternalInput").ap()
        return din[name]

    x_in = inp("x", (N, D))
    wq_in = inp("wq", (D, D))
    wk_in = inp("wk", (D, D))
    wv_in = inp("wv", (D, D))
    wo_in = inp("wo", (D, D))
    gw_in = inp("gw", (D, E))
    w1_in = inp("w1", (D, FF))
    b1_in = inp("b1", (FF,))
    w2_in = inp("w2", (FF, D))
    ident_in = inp("ident", (P, P))
    masks_in = inp("masks", (4, P, 512))
    if ln_affine:
        lng_in = inp("lng", (P, D))
        lnb_in = inp("lnb", (P, D))
    if use_qs:
        qs_in = inp("qs", (P, D))
    if use_gb:
        gb_in = inp("gb", (P, E))
    if use_b2:
        b2_in = inp("b2", (P, D))
    out_ext = nc.dram_tensor("out", [N // N_CORES, D], F32,
                             kind="ExternalOutput").ap()

    with tile.TileContext(nc) as tc:
        with (
            tc.tile_pool(name="persist", bufs=1) as pp,
            tc.tile_pool(name="dram", bufs=1, space="DRAM") as dp,
            tc.tile_pool(name="psum", bufs=8, space="PSUM") as psp,
            tc.tile_pool(name="stats", bufs=6) as stp,
            tc.tile_pool(name="work", bufs=2) as wp,
        ):
            def ps_tile(parts=P, n=512):
                return psp.tile([parts, n], F32, tag="ps", name="ps")

            # ---- persistent small tiles ----
            ident = pp.tile([P, P], F32, tag="ident")
            nc.sync.dma_start(ident[:], ident_in[:])
            c_eps5 = pp.tile([P, 1], F32, tag="c5")
            nc.vector.memset(c_eps5[:], 1e-5)
            c_eps12 = pp.tile([P, 1], F32, tag="c12")
            nc.vector.memset(c_eps12[:], 1e-12)
            c_m10 = pp.tile([P, 1], F32, tag="cm10")
            nc.vector.memset(c_m10[:], NEG_SHIFT)
            c_ones = pp.tile([P, TT * 4], F32, tag="ones")
            nc.vector.memset(c_ones[:], 1.0)
            gate_all = pp.tile([P, TT], F32, tag="gate")
            b1_sb = pp.tile([P, FC], F32, tag="b1")
            nc.sync.dma_start(b1_sb[:], b1_in.rearrange("(c p) -> p c", p=P))
            if ln_affine:
                lng = pp.tile([P, D], F32, tag="lng")
                lnb = pp.tile([P, D], F32, tag="lnb")
                nc.sync.dma_start(lng[:], lng_in[:])
                nc.sync.dma_start(lnb[:], lnb_in[:])
            if use_qs:
                qs = pp.tile([P, D], F32, tag="qs")
                nc.sync.dma_start(qs[:], qs_in[:])
            if use_gb:
                gbt = pp.tile([P, E], F32, tag="gb")
                nc.sync.dma_start(gbt[:], gb_in[:])
            if use_b2:
                b2t = pp.tile([P, D], F32, tag="b2")
                nc.sync.dma_start(b2t[:], b2_in[:])

            # ---- DRAM scratch ----
            oT_dram = dp.tile([D, N], F32R, tag="oT")
            rs_in = dp.tile([N, D], F32, tag="rsin")
            rs_out = dp.tile([N // N_CORES, D], F32, tag="rsout")

            def layernorm_tile(dst, src_sb, cols=D):
                """dst[P, cols] (any dtype) = LN(src_sb[P, cols]) along free dim."""
                s1 = stp.tile([P, 1], F32, tag="s1")
                nc.vector.reduce_sum(s1[:], src_sb[:], axis=AX.X)
                sq = wp.tile([P, cols], F32, tag="lnsq")
                ssq = stp.tile([P, 1], F32, tag="ssq")
                nc.scalar.activation(sq[:], src_sb[:], ACTF.Square,
                                     accum_out=ssq[:])
                mu = stp.tile([P, 1], F32, tag="mu")
                nc.vector.tensor_scalar_mul(mu[:], s1[:], 1.0 / cols)
                msq = stp.tile([P, 1], F32, tag="msq")
                nc.vector.tensor_scalar_mul(msq[:], ssq[:], 1.0 / cols)
                mu2 = stp.tile([P, 1], F32, tag="mu2")
                nc.vector.tensor_mul(mu2[:], mu[:], mu[:])
                var = stp.tile([P, 1], F32, tag="var")
                nc.vector.tensor_sub(var[:], msq[:], mu2[:])
                sd = stp.tile([P, 1], F32, tag="sd")
                nc.scalar.activation(sd[:], var[:], ACTF.Sqrt, bias=c_eps5[:])
                rstd = stp.tile([P, 1], F32, tag="rstd")
                nc.vector.reciprocal(rstd[:], sd[:])
                if ln_affine:
                    t = wp.tile([P, cols], F32, tag="lnt")
                    nc.vector.tensor_scalar(t[:], src_sb[:], mu[:], rstd[:],
                                            OP.subtract, OP.mult)
                    t2 = wp.tile([P, cols], F32, tag="lnt2")
                    nc.vector.tensor_mul(t2[:], t[:], lng[:, :cols])
                    nc.vector.tensor_add(dst[:], t2[:], lnb[:, :cols])
                else:
                    nc.vector.tensor_scalar(dst[:], src_sb[:], mu[:], rstd[:],
                                            OP.subtract, OP.mult)

            # ================= Phase A: LN1 + transpose -> hT =================
            with tc.tile_pool(name="phA", bufs=3) as pa:
                hT = pp.tile([P, DC, N], F32R, tag="hT")
                for t in range(TT):
                    x_t = pa.tile([P, D], F32, tag="xt")
                    nc.sync.dma_start(x_t[:], x_in[t * P:(t + 1) * P, :])
                    h_t = pa.tile([P, D], F32, tag="ht")
                    layernorm_tile(h_t, x_t)
                    for dc in range(DC):
                        ptp = ps_tile(P, P)
                        nc.tensor.transpose(ptp[:], h_t[:, dc * P:(dc + 1) * P],
                                            ident[:])
                        nc.any.tensor_copy(hT[:, dc, t * P:(t + 1) * P], ptp[:])

            # ================= Phase B: QKV + attention (4 col-blocks) ========
            NCB = 4          # col-blocks
            CBW = D // NCB   # 256 cols = 4 heads per block
            HPB = CBW // DH  # 4 heads per block
            pbm_ctx = tc.tile_pool(name="phBmask", bufs=1)
            pbm = pbm_ctx.__enter__()
            masks = pbm.tile([P, 4, 512], F32, tag="masks")
            for j in range(4):
                nc.sync.dma_start(masks[:, j, :], masks_in[j])
            for cb in range(NCB):
                with tc.tile_pool(name=f"phB{cb}", bufs=1) as pb:
                    wqs = pb.tile([P, DC, CBW], F32R, tag="wq")
                    wks = pb.tile([P, DC, CBW], F32R, tag="wk")
                    wvs = pb.tile([P, DC, CBW], F32R, tag="wv")
                    for w_sb, w_d in ((wqs, wq_in), (wks, wk_in), (wvs, wv_in)):
                        nc.sync.dma_start(
                            w_sb[:],
                            w_d.rearrange("(dc p) c -> p dc c", p=P)
                            [:, :, cb * CBW:(cb + 1) * CBW].bitcast(F32R))
                    qT = pb.tile([P, 2, N], F32R, tag="qT")
                    kT = pb.tile([P, 2, N], F32R, tag="kT")
                    v_aug = pb.tile([P, TT, HPB * 65], F32R, tag="vaug")
                    nc.vector.tensor_copy(
                        v_aug[:].rearrange("p t (x u) -> p t x u",
                                           u=65)[:, :, :, 64:65],
                        c_ones[:].rearrange("p (t x) -> p t x",
                                            x=4).unsqueeze(3))

                    for t in range(TT):
                        # --- q, k, v for this token tile ---
                        def qkv_mm(w_sb):
                            ps = ps_tile(P, CBW)
                            for dc in range(DC):
                                nc.tensor.matmul(
                                    ps[:], hT[:, dc, t * P:(t + 1) * P],
                                    w_sb[:, dc, :],
                                    start=(dc == 0), stop=(dc == DC - 1))
                            return ps
                        ps_q = qkv_mm(wqs)
                        ps_k = qkv_mm(wks)
                        ps_v = qkv_mm(wvs)
                        for hl in range(HPB):
                            nc.any.tensor_copy(
                                v_aug[:, t, hl * 65:hl * 65 + 64],
                                ps_v[:, hl * DH:(hl + 1) * DH])

                        # --- l2norm along dh per head ---
                        def l2norm(ps_x, apply_qs):
                            sq = pb.tile([P, CBW], F32, tag="l2sq", bufs=2, name="sq")
                            nc.scalar.square(sq[:], ps_x[:])
                            ss = stp.tile([P, HPB], F32, tag="l2ss")
                            nc.vector.reduce_sum(
                                ss[:], sq[:].rearrange("p (h d) -> p h d", d=DH),
                                axis=AX.X)
                            sd = stp.tile([P, HPB], F32, tag="l2sd")
                            nc.scalar.activation(sd[:], ss[:], ACTF.Sqrt,
                                                 bias=c_eps12[:])
                            ri = stp.tile([P, HPB], F32, tag="l2ri")
                            nc.vector.reciprocal(ri[:], sd[:])
                            xn = pb.tile([P, CBW], F32, tag="l2xn", bufs=2, name="xn")
                            nc.vector.tensor_tensor(
                                xn[:].rearrange("p (h d) -> p h d", d=DH),
                                ps_x[:].rearrange("p (h d) -> p h d", d=DH),
                                ri[:].unsqueeze(2).broadcast_to([P, HPB, DH]),
                                OP.mult)
                            if apply_qs:
                                xn2 = pb.tile([P, CBW], F32, tag="l2xn2", bufs=2, name="xn2")
                                nc.vector.tensor_mul(
                                    xn2[:], xn[:],
                                    qs[:, cb * CBW:(cb + 1) * CBW])
                                return xn2
                            return xn
                        qn = l2norm(ps_q, False)
                        kn = l2norm(ps_k, use_qs)

                        # --- transpose qn, kn into qT/kT (feature-major) ---
                        for i in range(2):
                            for src, dstT in ((qn, qT), (kn, kT)):
                                ptp = ps_tile(P, P)
                                nc.tensor.transpose(
                                    ptp[:], src[:, i * P:(i + 1) * P], ident[:])
                                nc.any.tensor_copy(
                                    dstT[:, i, t * P:(t + 1) * P], ptp[:])

                    # --- attention per (batch, local head) ---
                    for b in range(B):
                        for hl in range(HPB):
                            pi = (hl % 2) * DH          # partition offset
                            fi = hl // 2                # free index in qT/kT
                            qTh = qT[pi:pi + DH, fi, b * S:(b + 1) * S]
                            kTh = kT[pi:pi + DH, fi, b * S:(b + 1) * S]
                            for qb in range(2):
                                n_kt = (qb + 1) * 4
                                ps_av = ps_tile(65, 512)
                                for kt in range(n_kt):
                                    ps_s = ps_tile(P, 512)
                                    nc.tensor.matmul(
                                        ps_s[:], kTh[:, kt * P:(kt + 1) * P],
                                        qTh[:, qb * 512:(qb + 1) * 512],
                                        start=True, stop=True)
                                    e_raw = pb.tile([P, 512], F32, tag="eraw", bufs=3, name="e_raw")
                                    nc.scalar.activation(
                                        e_raw[:], ps_s[:], ACTF.Exp,
                                        bias=c_m10[:], scale=QK_SCALE)
                                    e = pb.tile([P, 512], F32R, tag="e", bufs=3, name="e")
                                    j = kt - qb * 4
                                    if j >= 0:
                                        nc.vector.tensor_mul(
                                            e[:], e_raw[:], masks[:, j, :])
                                    else:
                                        nc.vector.tensor_copy(e[:], e_raw[:])
                                    nc.tensor.matmul(
                                        ps_av[:],
                                        v_aug[:, b * 8 + kt,
                                              hl * 65:hl * 65 + 65],
                                        e[:],
                                        start=(kt == 0), stop=(kt == n_kt - 1))
                                # normalize by denominator row and spill
                                r_row = stp.tile([1, 512], F32, tag="rrow")
                                nc.vector.reciprocal(r_row[:], ps_av[64:65, :])
                                pbc = pb.tile([DH, 512], F32, tag="pbc", bufs=2, name="pbc")
                                nc.gpsimd.partition_broadcast(pbc[:], r_row[:])
                                o_sl = pb.tile([DH, 512], F32R, tag="osl", bufs=2, name="o_sl")
                                nc.vector.tensor_mul(o_sl[:], ps_av[0:DH, :],
                                                     pbc[:])
                                nc.sync.dma_start(
                                    oT_dram[cb * CBW + hl * DH:
                                            cb * CBW + (hl + 1) * DH,
                                            b * S + qb * 512:
                                            b * S + (qb + 1) * 512],
                                    o_sl[:])

            pbm_ctx.__exit__(None, None, None)
            # ============ Phase C: Wo + LN2 + transpose -> h2T, gate ==========
            with tc.tile_pool(name="phC", bufs=1) as pc:
                h2T = pp.tile([P, DC, N], F32R, tag="hT", name="h2T")
                wo_sb = pc.tile([P, DC, D], F32R, tag="wo")
                nc.sync.dma_start(
                    wo_sb[:],
                    wo_in.rearrange("(dc p) c -> p dc c", p=P).bitcast(F32R))
                with tc.tile_pool(name="phCs", bufs=3) as pcs:
                    for tb in range(4):
                        ps_c = [[None, None] for _ in range(4)]
                        for ttl in range(4):
                            for db in range(2):
                                ps_c[ttl][db] = ps_tile(P, 512)
                        for hd in range(DC):
                            o_ch = pcs.tile([P, 512], F32R, tag="och")
                            nc.sync.dma_start(
                                o_ch[:],
                                oT_dram[hd * P:(hd + 1) * P,
                                        tb * 512:(tb + 1) * 512])
                            for ttl in range(4):
                                for db in range(2):
                                    nc.tensor.matmul(
                                        ps_c[ttl][db][:],
                                        o_ch[:, ttl * P:(ttl + 1) * P],
                                        wo_sb[:, hd, db * 512:(db + 1) * 512],
                                        start=(hd == 0), stop=(hd == DC - 1))
                        for ttl in range(4):
                            t = tb * 4 + ttl
                            h2_t = pcs.tile([P, D], F32, tag="h2t")
                            for db in range(2):
                                nc.any.tensor_copy(
                                    h2_t[:, db * 512:(db + 1) * 512],
                                    ps_c[ttl][db][:])
                            h2n = pcs.tile([P, D], F32, tag="h2n")
                            layernorm_tile(h2n, h2_t)
                            for dc in range(DC):
                                ptp = ps_tile(P, P)
                                nc.tensor.transpose(
                                    ptp[:], h2n[:, dc * P:(dc + 1) * P],
                                    ident[:])
                                nc.any.tensor_copy(
                                    h2T[:, dc, t * P:(t + 1) * P], ptp[:])
                # gate: softmax(h2 @ gw)[:, 0] (gw permuted so col 0 = expert)
                gw_sb = pc.tile([P, DC, E], F32R, tag="gw")
                nc.sync.dma_start(
                    gw_sb[:],
                    gw_in.rearrange("(dc p) e -> p dc e", p=P).bitcast(F32R))
                for t in range(TT):
                    ps_g = ps_tile(P, E)
                    for dc in range(DC):
                        nc.tensor.matmul(ps_g[:],
                                         h2T[:, dc, t * P:(t + 1) * P],
                                         gw_sb[:, dc, :],
                                         start=(dc == 0), stop=(dc == DC - 1))
                    zg = stp.tile([P, E], F32, tag="zg")
                    if use_gb:
                        nc.vector.tensor_add(zg[:], ps_g[:], gbt[:])
                    else:
                        nc.vector.tensor_copy(zg[:], ps_g[:])
                    mx = stp.tile([P, 1], F32, tag="gmx")
                    nc.vector.reduce_max(mx[:], zg[:], axis=AX.X)
                    nmx = stp.tile([P, 1], F32, tag="gnmx")
                    nc.vector.tensor_scalar_mul(nmx[:], mx[:], -1.0)
                    eg = stp.tile([P, E], F32, tag="geg")
                    nc.scalar.activation(eg[:], zg[:], ACTF.Exp, bias=nmx[:])
                    sg = stp.tile([P, 1], F32, tag="gsg")
                    nc.vector.reduce_sum(sg[:], eg[:], axis=AX.X)
                    rg = stp.tile([P, 1], F32, tag="grg")
                    nc.vector.reciprocal(rg[:], sg[:])
                    nc.vector.tensor_mul(gate_all[:, t:t + 1], eg[:, 0:1],
                                         rg[:])

            # ================= Phase D: MoE (expert-local) ====================
            with tc.tile_pool(name="phD", bufs=1) as pd, \
                 tc.tile_pool(name="phDw", bufs=3) as pdw, \
                 tc.tile_pool(name="phDo", bufs=3) as pdo:
                for tb in range(4):
                    tok = slice(tb * 512, (tb + 1) * 512)
                    hidT = pd.tile([P, FC, 512], F32R, tag="hidT")
                    # D1: hid = gelu(h2 @ w1 + b1), feature-major
                    for fp in range(16):
                        w1p = pdw.tile([P, DC, 256], F32R, tag="w1p")
                        nc.sync.dma_start(
                            w1p[:],
                            w1_in.rearrange("(dc p) f -> p dc f", p=P)
                            [:, :, fp * 256:(fp + 1) * 256].bitcast(F32R))
                        for f2 in range(2):
                            fc = fp * 2 + f2
                            ps1 = ps_tile(P, 512)
                            for dc in range(DC):
                                nc.tensor.matmul(
                                    ps1[:], w1p[:, dc, f2 * P:(f2 + 1) * P],
                                    h2T[:, dc, tok],
                                    start=(dc == 0), stop=(dc == DC - 1))
                            nc.scalar.activation(
                                hidT[:, fc, :], ps1[:], ACTF.Gelu_apprx_tanh,
                                bias=b1_sb[:, fc:fc + 1])
                    # D2: out = (hid @ w2) * gate, token-major
                    ps2 = [[None, None] for _ in range(4)]
                    for ttl in range(4):
                        for db in range(2):
                            ps2[ttl][db] = ps_tile(P, 512)
                    for fc in range(FC):
                        w2p = pdw.tile([P, D], F32R, tag="w2p")
                        nc.sync.dma_start(
                            w2p[:], w2_in[fc * P:(fc + 1) * P, :].bitcast(F32R))
                        for ttl in range(4):
                            for db in range(2):
                                nc.tensor.matmul(
                                    ps2[ttl][db][:],
                                    hidT[:, fc, ttl * P:(ttl + 1) * P],
                                    w2p[:, db * 512:(db + 1) * 512],
                                    start=(fc == 0), stop=(fc == FC - 1))
                    for ttl in range(4):
                        t = tb * 4 + ttl
                        o_t = pdo.tile([P, D], F32, tag="ot")
                        for db in range(2):
                            nc.vector.tensor_scalar_mul(
                                o_t[:, db * 512:(db + 1) * 512],
                                ps2[ttl][db][:], gate_all[:, t:t + 1])
                        if use_b2:
                            nc.vector.tensor_add(o_t[:], o_t[:], b2t[:])
                        nc.sync.dma_start(rs_in[t * P:(t + 1) * P, :], o_t[:])

            # ================= Phase E: ReduceScatter + final LN ==============
            nc.gpsimd.collective_compute(
                "ReduceScatter", OP.add,
                replica_groups=[list(range(N_CORES))],
                ins=[rs_in[:]], outs=[rs_out[:]])
            with tc.tile_pool(name="phE", bufs=2) as pe:
                for t in range(N // N_CORES // P):   # 2 tiles
                    m_t = pe.tile([P, D], F32, tag="mt")
                    nc.sync.dma_start(m_t[:], rs_out[t * P:(t + 1) * P, :])
                    y_t = pe.tile([P, D], F32, tag="yt")
                    layernorm_tile(y_t, m_t)
                    nc.sync.dma_start(out_ext[t * P:(t + 1) * P, :], y_t[:])

    nc.compile()
    return nc


def kernel(**inputs):
    x = np.ascontiguousarray(inputs["x"], dtype=np.float32).reshape(N, D)
    ln_g = np.asarray(inputs["ln_g"], np.float32)
    ln_b = np.asarray(inputs["ln_b"], np.float32)
    Wq = np.ascontiguousarray(inputs["Wq"], np.float32)
    Wk = np.ascontiguousarray(inputs["Wk"], np.float32)
    Wv = np.ascontiguousarray(inputs["Wv"], np.float32)
    q_scale = np.asarray(inputs["q_scale"], np.float32)
    k_scale = np.asarray(inputs["k_scale"], np.float32)
    Wo = np.ascontiguousarray(inputs["Wo"], np.float32)
    gate_w = np.asarray(inputs["gate_w"], np.float32)
    gate_b = np.asarray(inputs["gate_b"], np.float32)
    w1 = np.asarray(inputs["w1"], np.float32)
    b1 = np.asarray(inputs["b1"], np.float32)
    w2 = np.asarray(inputs["w2"], np.float32)
    b2 = np.asarray(inputs["b2"], np.float32)

    ln_affine = not (np.all(ln_g == 1.0) and np.all(ln_b == 0.0))
    qs_flat = (q_scale.reshape(H, DH) * k_scale.reshape(H, DH)).reshape(D)
    use_qs = not np.all(qs_flat == 1.0)
    use_gb = not np.all(gate_b == 0.0)
    use_b2 = not np.all(b2 == 0.0)
    flags = (ln_affine, use_qs, use_gb, use_b2)

    if flags not in _CACHE:
        _CACHE[flags] = _build(flags)
    nc = _CACHE[flags]

    # causal masks for the 4 diagonal offsets of a [128k x 512q] block
    kk = np.arange(P)[:, None]
    qq = np.arange(512)[None, :]
    masks = np.stack([(qq >= kk + j * P).astype(np.float32) for j in range(4)])

    common = {
        "x": x, "wq": Wq, "wk": Wk, "wv": Wv, "wo": Wo,
        "b1_": None,  # placeholder removed below
        "ident": np.eye(P, dtype=np.float32),
        "masks": masks,
    }
    del common["b1_"]
    if ln_affine:
        common["lng"] = np.broadcast_to(ln_g, (P, D)).copy()
        common["lnb"] = np.broadcast_to(ln_b, (P, D)).copy()
    if use_qs:
        common["qs"] = np.broadcast_to(qs_flat, (P, D)).copy()

    in_maps = []
    for c in range(N_CORES):
        perm = [c] + [e for e in range(E) if e != c]
        m = dict(common)
        m["gw"] = np.ascontiguousarray(gate_w[:, perm])
        m["w1"] = np.ascontiguousarray(w1[c])
        m["b1"] = np.ascontiguousarray(b1[c])
        m["w2"] = np.ascontiguousarray(w2[c])
        if use_gb:
            m["gb"] = np.broadcast_to(gate_b[perm], (P, E)).copy()
        if use_b2:
            m["b2"] = np.broadcast_to(b2[c], (P, D)).copy()
        in_maps.append(m)

    global _last_in_maps
    _last_in_maps = in_maps
    res = run_bass_kernel_spmd(nc, in_maps, core_ids=list(range(N_CORES)))
    shards = [res.results[c]["out"] for c in range(N_CORES)]
    return np.concatenate(shards, axis=0).reshape(B, S, D)


# revision 10
# speedup vs baseline: 1.0232x; 1.0232x over previous
"""Trainium2 Bass kernel for nn_DenseEncoderLayer (qk-norm attention + dense MoE).

Sharding: all 8 cores run one SPMD program. Attention + norms are computed
redundantly on every core; the MoE expert dimension (E=8) is sharded one
expert per core (gate columns permuted per-core so column 0 is the local
expert). The gate-weighted expert partials are combined with an on-device
ReduceScatter over the token axis; each core applies the final LayerNorm to
its token shard and the host concatenates the shards.

Matmuls run as float32r (full-rate fp32 on the PE array).
"""
import numpy as np

import concourse.bass as bass
import concourse.tile as tile
from concourse import bacc, mybir
from concourse.bass_utils import run_bass_kernel_spmd

F32 = mybir.dt.float32
F32R = mybir.dt.float32r
AX = mybir.AxisListType
OP = mybir.AluOpType
ACTF = mybir.ActivationFunctionType

# Problem dims (hardcoded per spec)
B, S, D, H, DH, E, FF = 2, 1024, 1024, 16, 64, 8, 4096
N = B * S                 # 2048 tokens
P = 128                   # partitions
TT = N // P               # 16 token tiles
DC = D // P               # 8 d-chunks
FC = FF // P              # 32 ff-chunks
N_CORES = 8
QK_SCALE = 10.0
NEG_SHIFT = -10.0         # exp(QK_SCALE*logit - 10); logits in [-1, 1]

_CACHE = {}


def _build(flags):
    """Build the SPMD Bass program. flags: (ln_affine, qs, gate_b, b2)"""
    ln_affine, use_qs, use_gb, use_b2 = flags
    nc = bacc.Bacc("TRN2", target_bir_lowering=False, debug=False,
                   enable_asserts=False, num_devices=N_CORES)

    din = {}
    def inp(name, shape):
        din[name] = nc.dram_tensor(name, list(shape), F32, kind="ExternalInput").ap()
        return din[name]

    x_in = inp("x", (N, D))
    wq_in = inp("wq", (D, D))
    wk_in = inp("wk", (D, D))
    wv_in = inp("wv", (D, D))
    wo_in = inp("wo", (D, D))
    gw_in = inp("gw", (D, E))
    w1_in = inp("w1", (D, FF))
    b1_in = inp("b1", (FF,))
    w2_in = inp("w2", (FF, D))
    ident_in = inp("ident", (P, P))
    masks_in = inp("masks", (4, P, 512))
    if ln_affine:
        lng_in = inp("lng", (P, D))
        lnb_in = inp("lnb", (P, D))
    if use_qs:
        qs_in = inp("qs", (P, D))
    if use_gb:
        gb_in = inp("gb", (P, E))
    if use_b2:
        b2_in = inp("b2", (P, D))
    out_ext = nc.dram_tensor("out", [N // N_CORES, D], F32,
                             kind="ExternalOutput").ap()

    with tile.TileContext(nc) as tc:
        with (
            tc.tile_pool(name="persist", bufs=1) as pp,
            tc.tile_pool(name="dram", bufs=1, space="DRAM") as dp,
            tc.tile_pool(name="psum", bufs=8, space="PSUM") as psp,
            tc.tile_pool(name="stats", bufs=6) as stp,
            tc.tile_pool(name="work", bufs=2) as wp,
        ):
            def ps_tile(parts=P, n=512):
                return psp.tile([parts, n], F32, tag="ps", name="ps")

            # ---- persistent small tiles ----
            ident = pp.tile([P, P], F32, tag="ident")
            nc.sync.dma_start(ident[:], ident_in[:])
            c_eps5 = pp.tile([P, 1], F32, tag="c5")
            nc.vector.memset(c_eps5[:], 1e-5)
            c_eps12 = pp.tile([P, 1], F32, tag="c12")
            nc.vector.memset(c_eps12[:], 1e-12)
            c_m10 = pp.tile([P, 1], F32, tag="cm10")
            nc.vector.memset(c_m10[:], NEG_SHIFT)
            c_ones = pp.tile([P, TT * 4], F32, tag="ones")
            nc.vector.memset(c_ones[:], 1.0)
            gate_all = pp.tile([P, TT], F32, tag="gate")
            b1_sb = pp.tile([P, FC], F32, tag="b1")
            nc.sync.dma_start(b1_sb[:], b1_in.rearrange("(c p) -> p c", p=P))
            if ln_affine:
                lng = pp.tile([P, D], F32, tag="lng")
                lnb = pp.tile([P, D], F32, tag="lnb")
                nc.sync.dma_start(lng[:], lng_in[:])
                nc.sync.dma_start(lnb[:], lnb_in[:])
            if use_qs:
                qs = pp.tile([P, D], F32, tag="qs")
                nc.sync.dma_start(qs[:], qs_in[:])
            if use_gb:
                gbt = pp.tile([P, E], F32, tag="gb")
                nc.sync.dma_start(gbt[:], gb_in[:])
            if use_b2:
                b2t = pp.tile([P, D], F32, tag="b2")
                nc.sync.dma_start(b2t[:], b2_in[:])

            # ---- DRAM scratch ----
            oT_dram = dp.tile([D, N], F32R, tag="oT")
            rs_in = dp.tile([N, D], F32, tag="rsin")
            rs_out = dp.tile([N // N_CORES, D], F32, tag="rsout")

            def layernorm_tile(dst, src_sb, cols=D):
                """dst[P, cols] (any dtype) = LN(src_sb[P, cols]) along free dim."""
                s1 = stp.tile([P, 1], F32, tag="s1")
                nc.vector.reduce_sum(s1[:], src_sb[:], axis=AX.X)
                sq = wp.tile([P, cols], F32, tag="lnsq")
                ssq = stp.tile([P, 1], F32, tag="ssq")
                nc.scalar.activation(sq[:], src_sb[:], ACTF.Square,
                                     accum_out=ssq[:])
                mu = stp.tile([P, 1], F32, tag="mu")
                nc.vector.tensor_scalar_mul(mu[:], s1[:], 1.0 / cols)
                msq = stp.tile([P, 1], F32, tag="msq")
                nc.vector.tensor_scalar_mul(msq[:], ssq[:], 1.0 / cols)
                mu2 = stp.tile([P, 1], F32, tag="mu2")
                nc.vector.tensor_mul(mu2[:], mu[:], mu[:])
                var = stp.tile([P, 1], F32, tag="var")
                nc.vector.tensor_sub(var[:], msq[:], mu2[:])
                sd = stp.tile([P, 1], F32, tag="sd")
                nc.scalar.activation(sd[:], var[:], ACTF.Sqrt, bias=c_eps5[:])
                rstd = stp.tile([P, 1], F32, tag="rstd")
                nc.vector.reciprocal(rstd[:], sd[:])
                if ln_affine:
                    t = wp.tile([P, cols], F32, tag="lnt")
                    nc.vector.tensor_scalar(t[:], src_sb[:], mu[:], rstd[:],
                                            OP.subtract, OP.mult)
                    t2 = wp.tile([P, cols], F32, tag="lnt2")
                    nc.vector.tensor_mul(t2[:], t[:], lng[:, :cols])
                    nc.vector.tensor_add(dst[:], t2[:], lnb[:, :cols])
                else:
                    nc.vector.tensor_scalar(dst[:], src_sb[:], mu[:], rstd[:],
                                            OP.subtract, OP.mult)

            # ================= Phase A: LN1 + transpose -> hT =================
            with tc.tile_pool(name="phA", bufs=3) as pa:
                hT = pp.tile([P, DC, N], F32R, tag="hT")
                for t in range(TT):
                    x_t = pa.tile([P, D], F32, tag="xt")
                    nc.sync.dma_start(x_t[:], x_in[t * P:(t + 1) * P, :])
                    h_t = pa.tile([P, D], F32, tag="ht")
                    layernorm_tile(h_t, x_t)
                    for dc in range(DC):
                        ptp = ps_tile(P, P)
                        nc.tensor.transpose(ptp[:], h_t[:, dc * P:(dc + 1) * P],
                                            ident[:])
                        nc.any.tensor_copy(hT[:, dc, t * P:(t + 1) * P], ptp[:])

            # ================= Phase B: QKV + attention (4 col-blocks) ========
            NCB = 4          # col-blocks
            CBW = D // NCB   # 256 cols = 4 heads per block
            HPB = CBW // DH  # 4 heads per block
            pbm_ctx = tc.tile_pool(name="phBmask", bufs=1)
            pbm = pbm_ctx.__enter__()
            masks = pbm.tile([P, 4, 512], F32, tag="masks")
            for j in range(4):
                nc.sync.dma_start(masks[:, j, :], masks_in[j])
            for cb in range(NCB):
                with tc.tile_pool(name=f"phB{cb}", bufs=1) as pb:
                    wqs = pb.tile([P, DC, CBW], F32R, tag="wq")
                    wks = pb.tile([P, DC, CBW], F32R, tag="wk")
                    wvs = pb.tile([P, DC, CBW], F32R, tag="wv")
                    for w_sb, w_d in ((wqs, wq_in), (wks, wk_in), (wvs, wv_in)):
                        nc.sync.dma_start(
                            w_sb[:],
                            w_d.rearrange("(dc p) c -> p dc c", p=P)
                            [:, :, cb * CBW:(cb + 1) * CBW].bitcast(F32R))
                    qT = pb.tile([P, 2, N], F32R, tag="qT")
                    kT = pb.tile([P, 2, N], F32R, tag="kT")
                    v_aug = pb.tile([P, TT, HPB * 65], F32R, tag="vaug")
                    nc.vector.tensor_copy(
                        v_aug[:].rearrange("p t (x u) -> p t x u",
                                           u=65)[:, :, :, 64:65],
                        c_ones[:].rearrange("p (t x) -> p t x",
                                            x=4).unsqueeze(3))

                    for t in range(TT):
                        # --- q, k, v for this token tile ---
                        def qkv_mm(w_sb):
                            ps = ps_tile(P, CBW)
                            for dc in range(DC):
                                nc.tensor.matmul(
                                    ps[:], hT[:, dc, t * P:(t + 1) * P],
                                    w_sb[:, dc, :],
                                    start=(dc == 0), stop=(dc == DC - 1))
                            return ps
                        ps_q = qkv_mm(wqs)
                        ps_k = qkv_mm(wks)
                        ps_v = qkv_mm(wvs)
                        for hl in range(HPB):
                            nc.any.tensor_copy(
                                v_aug[:, t, hl * 65:hl * 65 + 64],
                                ps_v[:, hl * DH:(hl + 1) * DH])

                        # --- l2norm along dh per head ---
                        def l2norm(ps_x, apply_qs):
                            sq = pb.tile([P, CBW], F32, tag="l2sq", bufs=2, name="sq")
                            nc.scalar.square(sq[:], ps_x[:])
                            ss = stp.tile([P, HPB], F32, tag="l2ss")
                            nc.vector.reduce_sum(
                                ss[:], sq[:].rearrange("p (h d) -> p h d", d=DH),
                                axis=AX.X)
                            sd = stp.tile([P, HPB], F32, tag="l2sd")
                            nc.scalar.activation(sd[:], ss[:], ACTF.Sqrt,
                                                 bias=c_eps12[:])
                            ri = stp.tile([P, HPB], F32, tag="l2ri")
                            nc.vector.reciprocal(ri[:], sd[:])
                            xn = pb.tile([P, CBW], F32, tag="l2xn", bufs=2, name="xn")
                            nc.vector.tensor_tensor(
                                xn[:].rearrange("p (h d) -> p h d", d=DH),
                                ps_x[:].rearrange("p (h d) -> p h d", d=DH),
                                ri[:].unsqueeze(2).broadcast_to([P, HPB, DH]),
                                OP.mult)
                            if apply_qs:
                                xn2 = pb.tile([P, CBW], F32, tag="l2xn2", bufs=2, name="xn2")
                                nc.vector.tensor_mul(
                                    xn2[:], xn[:],
                                    qs[:, cb * CBW:(cb + 1) * CBW])
                                return xn2
                            return xn
                        qn = l2norm(ps_q, False)
                        kn = l2norm(ps_k, use_qs)

                        # --- transpose qn, kn into qT/kT (feature-major) ---
                        for i in range(2):
                            for src, dstT in ((qn, qT), (kn, kT)):
                                ptp = ps_tile(P, P)
                                nc.tensor.transpose(
                                    ptp[:], src[:, i * P:(i + 1) * P], ident[:])
                                nc.any.tensor_copy(
                                    dstT[:, i, t * P:(t + 1) * P], ptp[:])

                    # --- attention per (batch, local head) ---
                    for b in range(B):
                        for hl in range(HPB):
                            pi = (hl % 2) * DH          # partition offset
                            fi = hl // 2                # free index in qT/kT
                            qTh = qT[pi:pi + DH, fi, b * S:(b + 1) * S]
                            kTh = kT[pi:pi + DH, fi, b * S:(b + 1) * S]
                            for qb in range(2):
                                n_kt = (qb + 1) * 4
                                ps_av = ps_tile(65, 512)
                                LAG = 2
                                e_q = []

                                def mk_e(kt):
                                    ps_s = ps_tile(P, 512)
                                    nc.tensor.matmul(
                                        ps_s[:], kTh[:, kt * P:(kt + 1) * P],
                                        qTh[:, qb * 512:(qb + 1) * 512],
                                        start=True, stop=True)
                                    e_raw = pb.tile([P, 512], F32, tag="eraw", bufs=4, name="e_raw")
                                    nc.scalar.activation(
                                        e_raw[:], ps_s[:], ACTF.Exp,
                                        bias=c_m10[:], scale=QK_SCALE)
                                    e = pb.tile([P, 512], F32R, tag="e", bufs=4, name="e")
                                    j = kt - qb * 4
                                    if j >= 0:
                                        nc.vector.tensor_mul(
                                            e[:], e_raw[:], masks[:, j, :])
                                    else:
                                        nc.vector.tensor_copy(e[:], e_raw[:])
                                    return e

                                def do_av(kt, e):
                                    nc.tensor.matmul(
                                        ps_av[:],
                                        v_aug[:, b * 8 + kt,
                                              hl * 65:hl * 65 + 65],
                                        e[:],
                                        start=(kt == 0), stop=(kt == n_kt - 1))

                                for kt in range(n_kt):
                                    e_q.append(mk_e(kt))
                                    if kt >= LAG:
                                        do_av(kt - LAG, e_q[kt - LAG])
                                for kt in range(max(0, n_kt - LAG), n_kt):
                                    do_av(kt, e_q[kt])
                                # normalize by denominator row and spill
                                r_row = stp.tile([1, 512], F32, tag="rrow")
                                nc.vector.reciprocal(r_row[:], ps_av[64:65, :])
                                pbc = pb.tile([DH, 512], F32, tag="pbc", bufs=2, name="pbc")
                                nc.gpsimd.partition_broadcast(pbc[:], r_row[:])
                                o_sl = pb.tile([DH, 512], F32R, tag="osl", bufs=2, name="o_sl")
                                nc.vector.tensor_mul(o_sl[:], ps_av[0:DH, :],
                                                     pbc[:])
                                nc.sync.dma_start(
                                    oT_dram[cb * CBW + hl * DH:
                                            cb * CBW + (hl + 1) * DH,
                                            b * S + qb * 512:
                                            b * S + (qb + 1) * 512],
                                    o_sl[:])

            pbm_ctx.__exit__(None, None, None)
            # ============ Phase C: Wo + LN2 + transpose -> h2T, gate ==========
            with tc.tile_pool(name="phC", bufs=1) as pc:
                h2T = pp.tile([P, DC, N], F32R, tag="hT", name="h2T")
                wo_sb = pc.tile([P, DC, D], F32R, tag="wo")
                nc.sync.dma_start(
                    wo_sb[:],
                    wo_in.rearrange("(dc p) c -> p dc c", p=P).bitcast(F32R))
                with tc.tile_pool(name="phCs", bufs=3) as pcs:
                    for tb in range(4):
                        ps_c = [[None, None] for _ in range(4)]
                        for ttl in range(4):
                            for db in range(2):
                                ps_c[ttl][db] = ps_tile(P, 512)
                        for hd in range(DC):
                            o_ch = pcs.tile([P, 512], F32R, tag="och")
                            nc.sync.dma_start(
                                o_ch[:],
                                oT_dram[hd * P:(hd + 1) * P,
                                        tb * 512:(tb + 1) * 512])
                            for ttl in range(4):
                                for db in range(2):
                                    nc.tensor.matmul(
                                        ps_c[ttl][db][:],
                                        o_ch[:, ttl * P:(ttl + 1) * P],
                                        wo_sb[:, hd, db * 512:(db + 1) * 512],
                                        start=(hd == 0), stop=(hd == DC - 1))
                        for ttl in range(4):
                            t = tb * 4 + ttl
                            h2_t = pcs.tile([P, D], F32, tag="h2t")
                            for db in range(2):
                                nc.any.tensor_copy(
                                    h2_t[:, db * 512:(db + 1) * 512],
                                    ps_c[ttl][db][:])
                            h2n = pcs.tile([P, D], F32, tag="h2n")
                            layernorm_tile(h2n, h2_t)
                            for dc in range(DC):
                                ptp = ps_tile(P, P)
                                nc.tensor.transpose(
                                    ptp[:], h2n[:, dc * P:(dc + 1) * P],
                                    ident[:])
                                nc.any.tensor_copy(
                                    h2T[:, dc, t * P:(t + 1) * P], ptp[:])
                # gate: softmax(h2 @ gw)[:, 0] (gw permuted so col 0 = expert)
                gw_sb = pc.tile([P, DC, E], F32R, tag="gw")
                nc.sync.dma_start(
                    gw_sb[:],
                    gw_in.rearrange("(dc p) e -> p dc e", p=P).bitcast(F32R))
                for t in range(TT):
                    ps_g = ps_tile(P, E)
                    for dc in range(DC):
                        nc.tensor.matmul(ps_g[:],
                                         h2T[:, dc, t * P:(t + 1) * P],
                                         gw_sb[:, dc, :],
                                         start=(dc == 0), stop=(dc == DC - 1))
                    zg = stp.tile([P, E], F32, tag="zg")
                    if use_gb:
                        nc.vector.tensor_add(zg[:], ps_g[:], gbt[:])
                    else:
                        nc.vector.tensor_copy(zg[:], ps_g[:])
                    mx = stp.tile([P, 1], F32, tag="gmx")
                    nc.vector.reduce_max(mx[:], zg[:], axis=AX.X)
                    nmx = stp.tile([P, 1], F32, tag="gnmx")
                    nc.vector.tensor_scalar_mul(nmx[:], mx[:], -1.0)
                    eg = stp.tile([P, E], F32, tag="geg")
                    nc.scalar.activation(eg[:], zg[:], ACTF.Exp, bias=nmx[:])
                    sg = stp.tile([P, 1], F32, tag="gsg")
                    nc.vector.reduce_sum(sg[:], eg[:], axis=AX.X)
                    rg = stp.tile([P, 1], F32, tag="grg")
                    nc.vector.reciprocal(rg[:], sg[:])
                    nc.vector.tensor_mul(gate_all[:, t:t + 1], eg[:, 0:1],
                                         rg[:])

            # ================= Phase D: MoE (expert-local) ====================
            with tc.tile_pool(name="phD", bufs=1) as pd, \
                 tc.tile_pool(name="phDw", bufs=3) as pdw, \
                 tc.tile_pool(name="phDo", bufs=3) as pdo:
                for tb in range(4):
                    tok = slice(tb * 512, (tb + 1) * 512)
                    hidT = pd.tile([P, FC, 512], F32R, tag="hidT")
                    # D1: hid = gelu(h2 @ w1 + b1), feature-major
                    for fp in range(16):
                        w1p = pdw.tile([P, DC, 256], F32R, tag="w1p")
                        nc.sync.dma_start(
                            w1p[:],
                            w1_in.rearrange("(dc p) f -> p dc f", p=P)
                            [:, :, fp * 256:(fp + 1) * 256].bitcast(F32R))
                        for f2 in range(2):
                            fc = fp * 2 + f2
                            ps1 = ps_tile(P, 512)
                            for dc in range(DC):
                                nc.tensor.matmul(
                                    ps1[:], w1p[:, dc, f2 * P:(f2 + 1) * P],
                                    h2T[:, dc, tok],
                                    start=(dc == 0), stop=(dc == DC - 1))
                            nc.scalar.activation(
                                hidT[:, fc, :], ps1[:], ACTF.Gelu_apprx_tanh,
                                bias=b1_sb[:, fc:fc + 1])
                    # D2: out = (hid @ w2) * gate, token-major
                    ps2 = [[None, None] for _ in range(4)]
                    for ttl in range(4):
                        for db in range(2):
                            ps2[ttl][db] = ps_tile(P, 512)
                    for fc in range(FC):
                        w2p = pdw.tile([P, D], F32R, tag="w2p")
                        nc.sync.dma_start(
                            w2p[:], w2_in[fc * P:(fc + 1) * P, :].bitcast(F32R))
                        for ttl in range(4):
                            for db in range(2):
                                nc.tensor.matmul(
                                    ps2[ttl][db][:],
                                    hidT[:, fc, ttl * P:(ttl + 1) * P],
                                    w2p[:, db * 512:(db + 1) * 512],
                                    start=(fc == 0), stop=(fc == FC - 1))
                    for ttl in range(4):
                        t = tb * 4 + ttl
                        o_t = pdo.tile([P, D], F32, tag="ot")
                        for db in range(2):
                            nc.vector.tensor_scalar_mul(
                                o_t[:, db * 512:(db + 1) * 512],
                                ps2[ttl][db][:], gate_all[:, t:t + 1])
                        if use_b2:
                            nc.vector.tensor_add(o_t[:], o_t[:], b2t[:])
                        nc.sync.dma_start(rs_in[t * P:(t + 1) * P, :], o_t[:])
                    nc.gpsimd.collective_compute(
                        "ReduceScatter", OP.add,
                        replica_groups=[list(range(N_CORES))],
                        ins=[rs_in[tb * 512:(tb + 1) * 512, :]],
                        outs=[rs_out[tb * 64:(tb + 1) * 64, :]])

            # ================= Phase E: final LN on RS shards =================
            # (4 ReduceScatters were issued inside phase D, one per tok-block;
            #  rank c's shard of block tb is rs_out[tb*64:(tb+1)*64, :])
            with tc.tile_pool(name="phE", bufs=2) as pe:
                for t in range(2):   # 2 tiles of 128 = 2 RS blocks each
                    m_t = pe.tile([P, D], F32, tag="mt")
                    nc.sync.dma_start(m_t[:], rs_out[t * P:(t + 1) * P, :])
                    y_t = pe.tile([P, D], F32, tag="yt")
                    layernorm_tile(y_t, m_t)
                    nc.sync.dma_start(out_ext[t * P:(t + 1) * P, :], y_t[:])

    nc.compile()
    return nc


def kernel(**inputs):
    x = np.ascontiguousarray(inputs["x"], dtype=np.float32).reshape(N, D)
    ln_g = np.asarray(inputs["ln_g"], np.float32)
    ln_b = np.asarray(inputs["ln_b"], np.float32)
    Wq = np.ascontiguousarray(inputs["Wq"], np.float32)
    Wk = np.ascontiguousarray(inputs["Wk"], np.float32)
    Wv = np.ascontiguousarray(inputs["Wv"], np.float32)
    q_scale = np.asarray(inputs["q_scale"], np.float32)
    k_scale = np.asarray(inputs["k_scale"], np.float32)
    Wo = np.ascontiguousarray(inputs["Wo"], np.float32)
    gate_w = np.asarray(inputs["gate_w"], np.float32)
    gate_b = np.asarray(inputs["gate_b"], np.float32)
    w1 = np.asarray(inputs["w1"], np.float32)
    b1 = np.asarray(inputs["b1"], np.float32)
    w2 = np.asarray(inputs["w2"], np.float32)
    b2 = np.asarray(inputs["b2"], np.float32)

    ln_affine = not (np.all(ln_g == 1.0) and np.all(ln_b == 0.0))
    qs_flat = (q_scale.reshape(H, DH) * k_scale.reshape(H, DH)).reshape(D)
    use_qs = not np.all(qs_flat == 1.0)
    use_gb = not np.all(gate_b == 0.0)
    use_b2 = not np.all(b2 == 0.0)
    flags = (ln_affine, use_qs, use_gb, use_b2)

    if flags not in _CACHE:
        _CACHE[flags] = _build(flags)
    nc = _CACHE[flags]

    # causal masks for the 4 diagonal offsets of a [128k x 512q] block
    kk = np.arange(P)[:, None]
    qq = np.arange(512)[None, :]
    masks = np.stack([(qq >= kk + j * P).astype(np.float32) for j in range(4)])

    common = {
        "x": x, "wq": Wq, "wk": Wk, "wv": Wv, "wo": Wo,
        "b1_": None,  # placeholder removed below
        "ident": np.eye(P, dtype=np.float32),
        "masks": masks,
    }
    del common["b1_"]
    if ln_affine:
        common["lng"] = np.broadcast_to(ln_g, (P, D)).copy()
        common["lnb"] = np.broadcast_to(ln_b, (P, D)).copy()
    if use_qs:
        common["qs"] = np.broadcast_to(qs_flat, (P, D)).copy()

    in_maps = []
    for c in range(N_CORES):
        perm = [c] + [e for e in range(E) if e != c]
        m = dict(common)
        m["gw"] = np.ascontiguousarray(gate_w[:, perm])
        m["w1"] = np.ascontiguousarray(w1[c])
        m["b1"] = np.ascontiguousarray(b1[c])
        m["w2"] = np.ascontiguousarray(w2[c])
        if use_gb:
            m["gb"] = np.broadcast_to(gate_b[perm], (P, E)).copy()
        if use_b2:
            m["b2"] = np.broadcast_to(b2[c], (P, D)).copy()
        in_maps.append(m)

    global _last_in_maps
    _last_in_maps = in_maps
    res = run_bass_kernel_spmd(nc, in_maps, core_ids=list(range(N_CORES)))
    # Per-block ReduceScatter layout: core c's shard row tb*64+j holds
    # global token row tb*512 + c*64 + j.
    out = np.empty((N, D), np.float32)
    for c in range(N_CORES):
        shard = res.results[c]["out"]
        for tb in range(4):
            out[tb * 512 + c * 64: tb * 512 + (c + 1) * 64] = \
                shard[tb * 64:(tb + 1) * 64]
    return out.reshape(B, S, D)
